# revision 1
# baseline (speedup 1.0000x reference)
"""ChunkFlowClassifier Trainium2 kernel.

Math (per sample, reference.py):
  L = sum(attention_mask); mid = L // 2
  first_pool  = mean(hidden[1:mid])        # [H]
  second_pool = mean(hidden[mid:L-1])      # [H]
  fh, sh = LN(first_pool), LN(second_pool)
  flow = [fh, sh, sh - fh]                 # [3H]
  out = gelu(gelu(flow @ W1 + b1) @ W2 + b2) @ W3 + b3   # [5]

Strategy: data-parallel over 8 NeuronCores (8 samples/core). Host packs
only the rows each sample actually uses (positions 1..L-2; lengths are
ragged, avg ~50% of S) into a dense fp16 buffer plus per-row 0/1 mask
columns that route each row into one of 16 (sample, half) accumulators.
The device streams the packed buffer and pools via PE matmuls
  psum[16, H] += mask_tile[128, 16].T @ x_tile[128, H]
then runs LayerNorm + the MLP on-chip once per core.

Host-side algebraic folds (exact, just reassociation):
  flow @ W1 = fh@(W1a - W1c) + sh@(W1b + W1c)        (W1 = [W1a; W1b; W1c])
  LN scale/shift:  (xhat*g + b) @ M = xhat @ (g[:,None]*M) + b @ M
so the device only needs xhat (plain normalize) and a folded
W1f[2H, 512] (fp16) + b1f[512].
"""

import numpy as np

B, S, H = 64, 2048, 768
NCORES = 8
SPC = 8            # samples per core
C = 2              # 128-row tiles per DMA chunk
XBUFS = 16          # SBUF double-buffering depth for the stream
ALT_ENGINE = True  # alternate x-chunk DMAs between the two HWDGE rings
XDT_NAME = "float16"  # dtype hidden is streamed in

_NC_CACHE = {}


def _build_nc(nchunk, repeat=1):
    """Build + compile the per-core Bass program for `nchunk` C-tile chunks.

    repeat > 1 wraps the streaming loop in a Tile For_i that re-streams the
    same data `repeat` times (used only for timing; output is unchanged).
    """
    import concourse.bacc as bacc
    import concourse.tile as tile
    from concourse import mybir

    dt = mybir.dt
    f32 = dt.float32
    xdt = getattr(dt, XDT_NAME)
    Alu = mybir.AluOpType
    Act = mybir.ActivationFunctionType

    NT = nchunk * C

    nc = bacc.Bacc("TRN2", target_bir_lowering=False, debug=False,
                   num_devices=NCORES)

    def din(name, shape, d=f32):
        return nc.dram_tensor(name, shape, d, kind="ExternalInput").ap()

    xin = din("xin", [nchunk, 128, C * H], xdt)
    mc = din("mc", [128, NT * 16], xdt)
    epsc = din("epsc", [16, 1])
    idn = din("idn", [16, 16])
    w1 = din("w1", [2 * H, 512], xdt)       # folded (see module docstring)
    b1 = din("b1", [1, 512], xdt)
    w2 = din("w2", [512, 128])
    b2 = din("b2", [1, 128])
    w3 = din("w3", [128, 5])
    b3 = din("b3", [1, 5])
    out = nc.dram_tensor("out", [SPC, 5], f32, kind="ExternalOutput").ap()

    with tile.TileContext(nc) as tc:
        with (
            tc.tile_pool(name="xp", bufs=XBUFS) as xp,
            tc.tile_pool(name="sg", bufs=1) as sg,
            tc.tile_pool(name="sm", bufs=1) as sm,
            tc.tile_pool(name="tpp", bufs=2, space="PSUM") as tpp,
            tc.tile_pool(name="mlp", bufs=1, space="PSUM") as mlp,
            tc.tile_pool(name="acc", bufs=1, space="PSUM") as acc,
        ):
            mc_sb = sg.tile([128, NT * 16], xdt)
            nc.sync.dma_start(out=mc_sb, in_=mc)
            # weights/constants: small now, prefetch alongside the stream
            w1_sb = sg.tile([128, 12, 512], xdt)
            nc.scalar.dma_start(out=w1_sb, in_=w1.rearrange("(k p) n -> p k n", p=128))
            w2_sb = sg.tile([128, 4, 128], f32)
            nc.scalar.dma_start(out=w2_sb, in_=w2.rearrange("(k p) n -> p k n", p=128))
            w3_sb = sg.tile([128, 5], f32)
            nc.scalar.dma_start(out=w3_sb, in_=w3)
            b1_sb = sm.tile([1, 512], xdt)
            nc.scalar.dma_start(out=b1_sb, in_=b1)
            b2_sb = sm.tile([1, 128], f32)
            nc.scalar.dma_start(out=b2_sb, in_=b2)
            b3_sb = sm.tile([1, 5], f32)
            nc.scalar.dma_start(out=b3_sb, in_=b3)
            epsc_sb = sm.tile([16, 1], f32)
            nc.sync.dma_start(out=epsc_sb, in_=epsc)
            idn_sb = sm.tile([16, 16], f32)
            nc.sync.dma_start(out=idn_sb, in_=idn)
            ones_sb = sm.tile([1, SPC], xdt)
            nc.vector.memset(ones_sb, 1.0)
            onesf_sb = sm.tile([1, SPC], f32)
            nc.vector.memset(onesf_sb, 1.0)
            scr_sb = sm.tile([1, 2], f32)
            nc.vector.memset(scr_sb, 1.0)
            # touch Sqrt+Gelu once early so ACT table loads overlap the stream
            nc.scalar.activation(out=scr_sb[:, 0:1], in_=scr_sb[:, 0:1],
                                 func=Act.Sqrt)
            nc.scalar.activation(out=scr_sb[:, 1:2], in_=scr_sb[:, 1:2],
                                 func=Act.Gelu)

            ps1 = acc.tile([16, 512], f32)
            ps2 = acc.tile([16, 256], f32)

            def stream_body(_i=None):
                for g in range(nchunk):
                    xt = xp.tile([128, C * H], xdt, tag="x")
                    eng = nc.sync if (g % 2 == 0 or not ALT_ENGINE) else nc.scalar
                    eng.dma_start(out=xt, in_=xin[g])
                    for c in range(C):
                        t = g * C + c
                        first = t == 0
                        last = t == NT - 1
                        lhs = mc_sb[:, t * 16:(t + 1) * 16]
                        nc.tensor.matmul(ps1, lhs, xt[:, c * H:c * H + 512],
                                         start=first, stop=last)
                        nc.tensor.matmul(ps2, lhs, xt[:, c * H + 512:(c + 1) * H],
                                         start=first, stop=last)

            def full_pass():
                stream_body()
                # LayerNorm directly on the raw sums: LN is scale-invariant,
                # with eps scaled by cnt^2 (host-provided) to stay exact.
                stats = sm.tile([16, 3, 6], f32)
                nc.vector.bn_stats(out=stats[:, 0, :], in_=ps1[:, 0:256])
                nc.vector.bn_stats(out=stats[:, 1, :], in_=ps1[:, 256:512])
                nc.vector.bn_stats(out=stats[:, 2, :], in_=ps2)
                mv = sm.tile([16, 2], f32)
                nc.vector.bn_aggr(out=mv, in_=stats)
                rstd = sm.tile([16, 1], f32)
                nc.scalar.activation(out=rstd, in_=mv[:, 1:2], func=Act.Sqrt,
                                     bias=epsc_sb, scale=1.0)
                nc.vector.reciprocal(out=rstd, in_=rstd)
                # keep PE busy through the LN chain so HAM stays at full clock
                warm = mlp.tile([16, 32], f32, tag="warm")
                nc.tensor.matmul(warm[:, 0:18], idn_sb, stats.rearrange("p a b -> p (a b)"),
                                 start=True, stop=True)
                nc.tensor.matmul(warm[:, 18:20], idn_sb, mv, start=True, stop=True)
                nc.tensor.matmul(warm[:, 20:21], idn_sb, rstd, start=True, stop=True)
                xn1 = sg.tile([16, 512], f32)
                xn2 = sg.tile([16, 256], f32)
                nc.vector.tensor_scalar(out=xn1, in0=ps1, scalar1=mv[:, 0:1],
                                        scalar2=rstd, op0=Alu.subtract, op1=Alu.mult)
                nc.vector.tensor_scalar(out=xn2, in0=ps2, scalar1=mv[:, 0:1],
                                        scalar2=rstd, op0=Alu.subtract, op1=Alu.mult)

                # transpose the 16 normalized vectors -> 12 k-tiles [128, 8] fp16
                flowT = sg.tile([128, 12, SPC], xdt)
                tp6 = tpp.tile([128, 6, 16], f32, tag="tp")
                for c6 in range(6):
                    src_ap = (xn1[:, c6 * 128:(c6 + 1) * 128] if c6 < 4
                              else xn2[:, (c6 - 4) * 128:(c6 - 3) * 128])
                    nc.tensor.matmul(tp6[:, c6, :], src_ap,
                                     idn_sb, start=True, stop=True)
                # tp6[:, c, h*8:h*8+8] holds (half h, chunk c); flowT k-tile
                # order is [fh chunks 0..5 | sh chunks 0..5]
                nc.vector.tensor_copy(flowT[:, 0:6, :], tp6[:, :, 0:SPC])
                nc.vector.tensor_copy(flowT[:, 6:12, :], tp6[:, :, SPC:16])

                # layer 1: h1[8, 512] = gelu(fh @ W1f[:H] + sh @ W1f[H:] + b1f)
                h1ps = mlp.tile([SPC, 512], f32, tag="h1")
                for k in range(12):
                    nc.tensor.matmul(h1ps, flowT[:, k, :], w1_sb[:, k, :],
                                     start=(k == 0), stop=False)
                nc.tensor.matmul(h1ps, ones_sb, b1_sb, start=False, stop=True)
                h1 = sg.tile([SPC, 512], f32)
                nc.scalar.activation(out=h1, in_=h1ps, func=Act.Gelu)

                h1T = sg.tile([128, 4, SPC], f32)
                tp4 = tpp.tile([128, 4, SPC], f32, tag="tp")
                for k in range(4):
                    nc.tensor.matmul(tp4[:, k, :], h1[:, k * 128:(k + 1) * 128],
                                     idn_sb[0:SPC, 0:SPC], start=True, stop=True)
                nc.vector.tensor_copy(h1T, tp4)

                # layer 2: h2[8, 128] = gelu(h1 @ W2 + b2)
                h2ps = mlp.tile([SPC, 128], f32, tag="h2")
                for k in range(4):
                    nc.tensor.matmul(h2ps, h1T[:, k, :], w2_sb[:, k, :],
                                     start=(k == 0), stop=False)
                nc.tensor.matmul(h2ps, onesf_sb, b2_sb, start=False, stop=True)
                h2 = sg.tile([SPC, 128], f32)
                nc.scalar.activation(out=h2, in_=h2ps, func=Act.Gelu)

                tp = tpp.tile([128, 16], f32, tag="tp")
                nc.tensor.matmul(tp[:, 0:SPC], h2, idn_sb[0:SPC, 0:SPC],
                                 start=True, stop=True)
                h2T = sg.tile([128, SPC], f32)
                nc.vector.tensor_copy(h2T, tp[:, 0:SPC])

                # layer 3: out[8, 5] = h2 @ W3 + b3
                ops = mlp.tile([SPC, 5], f32, tag="o")
                nc.tensor.matmul(ops, h2T, w3_sb, start=True, stop=False)
                nc.tensor.matmul(ops, onesf_sb, b3_sb, start=False, stop=True)
                o_sb = sm.tile([SPC, 5], f32)
                nc.vector.tensor_copy(o_sb, ops)
                nc.sync.dma_start(out=out, in_=o_sb)

            if repeat == 1:
                full_pass()
            else:
                unroll = globals().get("_TIMING_UNROLL", 1)
                with tc.For_i(0, repeat, 1) as _i:
                    for _u in range(unroll):
                        full_pass()

    nc.compile()
    return nc


def _get_nc(nchunk, repeat=1):
    key = (nchunk, repeat)
    if key not in _NC_CACHE:
        _NC_CACHE[key] = _build_nc(nchunk, repeat)
    return _NC_CACHE[key]


def _prepare(hidden, attention_mask, gamma, beta, W1, b1, W2, b2, W3, b3):
    """Host-side sharding + packing. Returns (in_maps, core_samples, nchunk)."""
    xdt = np.dtype(XDT_NAME)
    L = attention_mask.astype(np.int64).sum(1)          # [B]
    mid = L // 2
    rows = L - 2                                        # used rows per sample

    # balance total rows across cores (greedy LPT, exactly SPC samples/core)
    order = np.argsort(-rows, kind="stable")
    core_rows = [0] * NCORES
    core_samples = [[] for _ in range(NCORES)]
    for b in order:
        cands = sorted(range(NCORES),
                       key=lambda cc: (len(core_samples[cc]) >= SPC, core_rows[cc]))
        cc = cands[0]
        core_samples[cc].append(int(b))
        core_rows[cc] += int(rows[b])

    maxrows = max(core_rows)
    nchunk = max(1, -(-maxrows // (128 * C)))
    NT = nchunk * C
    R = NT * 128

    hidden2d = np.ascontiguousarray(hidden).reshape(B * S, H)
    gamma = np.asarray(gamma, np.float64)
    beta = np.asarray(beta, np.float64)
    W1 = np.asarray(W1, np.float64)
    b1 = np.asarray(b1, np.float64)
    W1a, W1b, W1c = W1[0:H], W1[H:2 * H], W1[2 * H:3 * H]
    W1f = np.concatenate([gamma[:, None] * (W1a - W1c),
                          gamma[:, None] * (W1b + W1c)], axis=0)
    b1f = b1 + beta @ (W1a + W1b)
    shared = dict(
        idn=np.eye(16, dtype=np.float32),
        w1=W1f.astype(xdt),
        b1=b1f.astype(xdt).reshape(1, -1),
        w2=np.ascontiguousarray(W2, np.float32),
        b2=np.ascontiguousarray(b2, np.float32).reshape(1, -1),
        w3=np.ascontiguousarray(W3, np.float32),
        b3=np.ascontiguousarray(b3, np.float32).reshape(1, -1),
    )

    in_maps = []
    for cc in range(NCORES):
        samples = core_samples[cc]
        rcounts = [int(rows[b]) for b in samples]
        Rc = sum(rcounts)
        idx = np.concatenate([b * S + np.arange(1, int(L[b]) - 1) for b in samples])
        packed = np.zeros((R, H), xdt)
        packed[:Rc] = hidden2d[idx]
        xin = np.ascontiguousarray(
            packed.reshape(nchunk, C, 128, H).transpose(0, 2, 1, 3)
            .reshape(nchunk, 128, C * H))

        pos = np.concatenate([np.arange(1, int(L[b]) - 1) for b in samples])
        sj = np.repeat(np.arange(SPC), rcounts)
        mids = np.repeat([int(mid[b]) for b in samples], rcounts)
        col = np.where(pos < mids, sj, sj + SPC)
        m = np.zeros((R, 16), xdt)
        m[np.arange(Rc), col] = 1.0
        mc = np.ascontiguousarray(
            m.reshape(NT, 128, 16).transpose(1, 0, 2).reshape(128, NT * 16))

        cnt1 = np.array([max(int(mid[b]) - 1, 1) for b in samples], np.float64)
        cnt2 = np.array([max(int(L[b]) - 1 - int(mid[b]), 1) for b in samples],
                        np.float64)
        epsc = np.concatenate([1e-5 * cnt1 ** 2, 1e-5 * cnt2 ** 2])
        epsc = epsc.astype(np.float32).reshape(16, 1)

        in_maps.append(dict(xin=xin, mc=mc, epsc=epsc, **shared))
    return in_maps, core_samples, nchunk


def kernel(**inputs):
    from concourse.bass_utils import run_bass_kernel_spmd

    args = {k: np.asarray(v) for k, v in inputs.items()}
    in_maps, core_samples, nchunk = _prepare(
        args["hidden"].astype(np.float32, copy=False),
        args["attention_mask"],
        args["gamma"], args["beta"],
        args["W1"], args["b1"], args["W2"], args["b2"], args["W3"], args["b3"],
    )
    nc = _get_nc(nchunk)
    res = run_bass_kernel_spmd(nc, in_maps, core_ids=list(range(NCORES)))
    out = np.zeros((B, 5), np.float32)
    for cc in range(NCORES):
        o = res.results[cc]["out"]
        for j, b in enumerate(core_samples[cc]):
            out[b] = o[j]
    return out



# revision 60
# speedup vs baseline: 600.7203x; 600.7203x over previous
"""ChunkFlowClassifier Trainium2 kernel.

Math (per sample, reference.py):
  L = sum(attention_mask); mid = L // 2
  first_pool  = mean(hidden[1:mid])        # [H]
  second_pool = mean(hidden[mid:L-1])      # [H]
  fh, sh = LN(first_pool), LN(second_pool)
  flow = [fh, sh, sh - fh]                 # [3H]
  out = gelu(gelu(flow @ W1 + b1) @ W2 + b2) @ W3 + b3   # [5]

Strategy: data-parallel over 8 NeuronCores (8 samples/core). Host packs
only the rows each sample actually uses (positions 1..L-2; lengths are
ragged, avg ~50% of S) into a dense fp16 buffer plus per-row 0/1 mask
columns that route each row into one of 8 per-half accumulators.
Rows are ordered [all first-half rows | all second-half rows] with a
core-common boundary tile tb, so the first-half pools close mid-stream:
their LayerNorm + L1 matmuls run hidden under the second half's DMA
stream; only the second half's tail is exposed after the stream.
The stream pools via PE matmuls
  psA/psB[8, H-chunk] += mask_tile[128, 8].T @ x_tile[128, H-chunk]
and the MLP runs transposed (h1T/h2T chunks [128, 8] in psum, weights
stationary) so gelu/copies act on 8-wide partitions-parallel tiles.
Each h1T chunk gets its own psum bank: interleaved multi-instruction
accumulation groups inside one bank corrupt all but the last region.

Host-side algebraic folds (exact, just reassociation):
  flow @ W1 = fh@(W1a - W1c) + sh@(W1b + W1c)        (W1 = [W1a; W1b; W1c])
  LN scale/shift:  (xhat*g + b) @ M = xhat @ (g[:,None]*M) + b @ M
  LN on raw sums (scale-invariant) with eps scaled by cnt^2
so the device only needs xhat (plain normalize) and a folded
W1f[2H, 512] (fp16) + b1f[512].
"""

import numpy as np

B, S, H = 64, 2048, 768
NCORES = 8
SPC = 8            # samples per core
C = 2              # 128-row tiles per DMA chunk
XBUFS = 16          # SBUF double-buffering depth for the stream
XDT_NAME = "float16"  # dtype hidden is streamed in

# Experiment knobs (cfg dict keys)
DEF_CFG = dict(
    stream_engines=("sync",),           # queues for x chunks (sync only: the
                                        # ACT queue stays free for tail ops)
    weights_engine="gpsimd",            # queue for weight/const prefetch
    tail="full",                        # "full" | "none" (stream only)
    acc_bufs=1,                         # psum accumulator double-buffering
    split=True,                         # pipeline first-half tail under stream
    emit_delay=2,                       # chunks after boundary before A-tail
)

_NC_CACHE = {}


def _build_nc(nchunk, repeat=1, cfg=None, tb=0):
    """Build + compile the per-core Bass program for `nchunk` C-tile chunks.

    repeat > 1 wraps the streaming loop in a Tile For_i that re-streams the
    same data `repeat` times (used only for timing; output is unchanged).
    """
    import concourse.bacc as bacc
    import concourse.tile as tile
    from concourse import mybir

    cfg = {**DEF_CFG, **(cfg or {})}

    dt = mybir.dt
    f32 = dt.float32
    xdt = getattr(dt, XDT_NAME)
    Alu = mybir.AluOpType
    Act = mybir.ActivationFunctionType

    NT = nchunk * C

    nc = bacc.Bacc("TRN2", target_bir_lowering=False, debug=False,
                   num_devices=NCORES)

    def din(name, shape, d=f32):
        return nc.dram_tensor(name, shape, d, kind="ExternalInput").ap()

    xin = din("xin", [nchunk, 128, C * H], xdt)
    mc = din("mc", [128, NT * SPC], xdt)
    epsc = din("epsc", [SPC, 2])
    idn = din("idn", [16, 16])
    w1 = din("w1", [2 * H, 512], xdt)       # folded (see module docstring)
    b1 = din("b1", [1, 512], xdt)
    w2 = din("w2", [512, 128], xdt)
    b2 = din("b2", [1, 128], xdt)
    w3 = din("w3", [128, 5], xdt)
    b3 = din("b3", [1, 5], xdt)
    out = nc.dram_tensor("out", [SPC, 5], f32, kind="ExternalOutput").ap()
    dbg = cfg.get("debug_outs")
    if dbg:
        dbg_out = {nm: nc.dram_tensor(f"dbg_{nm}", shp, f32,
                                      kind="ExternalOutput").ap()
                   for nm, shp in dbg.items()}

    weng = getattr(nc, cfg["weights_engine"])
    sengs = [getattr(nc, e) for e in cfg["stream_engines"]]

    with tile.TileContext(nc) as tc:
        with (
            tc.tile_pool(name="xp", bufs=XBUFS) as xp,
            tc.tile_pool(name="sg", bufs=1) as sg,
            tc.tile_pool(name="sm", bufs=1) as sm,
            tc.tile_pool(name="tpp", bufs=1, space="PSUM") as tpp,
            tc.tile_pool(name="mlp", bufs=1, space="PSUM") as mlp,
            tc.tile_pool(name="acc", bufs=cfg["acc_bufs"], space="PSUM") as acc,
        ):
            # mc/epsc/idn go on the scalar (ACT) queue: with the stream on
            # sync only, chunk 0's DMA is then first in its queue
            mc_sb = sg.tile([128, NT * SPC], xdt)
            nc.scalar.dma_start(out=mc_sb, in_=mc)
            # weights/constants: small now, prefetch alongside the stream
            w1_sb = sg.tile([128, 12, 512], xdt)
            weng.dma_start(out=w1_sb, in_=w1.rearrange("(k p) n -> p k n", p=128))
            w2_sb = sg.tile([128, 4, 128], xdt)
            weng.dma_start(out=w2_sb, in_=w2.rearrange("(k p) n -> p k n", p=128))
            w3_sb = sg.tile([128, 5], xdt)
            weng.dma_start(out=w3_sb, in_=w3)
            b1_sb = sm.tile([1, 512], xdt)
            weng.dma_start(out=b1_sb, in_=b1)
            b2_sb = sm.tile([1, 128], xdt)
            weng.dma_start(out=b2_sb, in_=b2)
            b3_sb = sm.tile([1, 5], xdt)
            weng.dma_start(out=b3_sb, in_=b3)
            epsc_sb = sm.tile([SPC, 2], f32)
            nc.scalar.dma_start(out=epsc_sb, in_=epsc)
            idn_sb = sm.tile([16, 16], f32)
            nc.scalar.dma_start(out=idn_sb, in_=idn)
            ones_sb = sm.tile([1, SPC], xdt)
            nc.vector.memset(ones_sb, 1.0)
            scr_sb = sm.tile([1, 2], f32)
            nc.vector.memset(scr_sb, 1.0)
            # pre-touch ACT tables: Gelu first, Sqrt LAST so the table set the
            # tail needs first (sqrt_and_others) is resident when the stream
            # ends; the only exposed reload is then Sqrt -> Gelu.
            nc.scalar.activation(out=scr_sb[:, 1:2], in_=scr_sb[:, 1:2],
                                 func=Act.Gelu)
            nc.scalar.activation(out=scr_sb[:, 0:1], in_=scr_sb[:, 0:1],
                                 func=Act.Sqrt)

            def stream_body(ps1, ps2):
                for g in range(nchunk):
                    xt = xp.tile([128, C * H], xdt, tag="x")
                    eng = sengs[g % len(sengs)]
                    eng.dma_start(out=xt, in_=xin[g])
                    for c in range(C):
                        t = g * C + c
                        first = t == 0
                        last = t == NT - 1
                        lhs = mc_sb[:, t * SPC:(t + 1) * SPC]
                        nc.tensor.matmul(ps1, lhs, xt[:, c * H:c * H + 512],
                                         start=first, stop=last)
                        nc.tensor.matmul(ps2, lhs, xt[:, c * H + 512:(c + 1) * H],
                                         start=first, stop=last)

            dbg_tiles = {}

            def half_ln(ps1h, ps2h, half, h1cs, tph):
                """LN + transpose + 24 L1 matmuls for one half (8 accumulators).

                half 0 = first halves (psA, W1f k-tiles 0..5),
                half 1 = second halves (psB, k-tiles 6..11)."""
                stats = sm.tile([SPC, 2, 6], f32, tag=f"st{half}")
                nc.vector.bn_stats(out=stats[:, 0, :], in_=ps1h)
                nc.vector.bn_stats(out=stats[:, 1, :], in_=ps2h)
                mv = sm.tile([SPC, 2], f32, tag=f"mv{half}")
                nc.vector.bn_aggr(out=mv, in_=stats)
                rstd = sm.tile([SPC, 1], f32, tag=f"rs{half}")
                nc.scalar.activation(out=rstd, in_=mv[:, 1:2], func=Act.Sqrt,
                                     bias=epsc_sb[:, half:half + 1], scale=1.0)
                nc.vector.reciprocal(out=rstd, in_=rstd)
                # -mean*rstd, so xn1 can run on ACT (Copy: scale*x+bias) while
                # DVE does xn2 — the two normalizes run concurrently
                nmr = sm.tile([SPC, 1], f32, tag=f"nm{half}")
                nc.vector.tensor_scalar(out=nmr, in0=mv[:, 0:1], scalar1=rstd,
                                        scalar2=-1.0, op0=Alu.mult, op1=Alu.mult)
                xn1 = sg.tile([SPC, 512], f32, tag=f"xn1{half}")
                xn2 = sg.tile([SPC, 256], f32, tag=f"xn2{half}")
                nc.scalar.activation(out=xn1, in_=ps1h, func=Act.Identity,
                                     bias=nmr, scale=rstd)
                nc.vector.tensor_scalar(out=xn2, in0=ps2h, scalar1=mv[:, 0:1],
                                        scalar2=rstd, op0=Alu.subtract, op1=Alu.mult)
                for c6 in range(6):
                    src_ap = (xn1[:, c6 * 128:(c6 + 1) * 128] if c6 < 4
                              else xn2[:, (c6 - 4) * 128:(c6 - 3) * 128])
                    nc.tensor.matmul(tph[:, c6, :], src_ap,
                                     idn_sb[0:SPC, 0:SPC], start=True, stop=True)
                flowTh = sg.tile([128, 6, SPC], xdt, tag=f"fT{half}")
                nc.vector.tensor_copy(flowTh, tph)
                dbg_tiles[f"xn1_{half}"] = xn1
                dbg_tiles[f"xn2_{half}"] = xn2
                dbg_tiles[f"fT{half}"] = flowTh
                dbg_tiles[f"ps1_{half}"] = ps1h
                dbg_tiles[f"mv_{half}"] = mv
                dbg_tiles[f"rstd_{half}"] = rstd
                # layer 1, transposed: h1T[c][128, 8] += W1f_kc^T-chunks @ xhatT
                # (each c-chunk must be its OWN psum tile/bank: interleaved
                # multi-instruction accumulation groups inside one bank corrupt
                # all but the last-started region)
                for k in range(6):
                    for cc4 in range(4):
                        nc.tensor.matmul(
                            h1cs[cc4],
                            w1_sb[:, 6 * half + k, cc4 * 128:(cc4 + 1) * 128],
                            flowTh[:, k, :],
                            start=(half == 0 and k == 0), stop=False)

            def split_stream(h1cs, tph):
                psA1 = acc.tile([SPC, 512], f32, tag="psA1")
                psB1 = acc.tile([SPC, 512], f32, tag="psB1")
                ps2x = acc.tile([SPC, 512], f32, tag="ps2x")
                psA2, psB2 = ps2x[:, 0:256], ps2x[:, 256:512]
                emit_at = (min(tb // C + cfg["emit_delay"], nchunk - 1)
                           if cfg["split"] else None)
                for g in range(nchunk):
                    xt = xp.tile([128, C * H], xdt, tag="x")
                    eng = sengs[g % len(sengs)]
                    eng.dma_start(out=xt, in_=xin[g])
                    for c in range(C):
                        t = g * C + c
                        lhs = mc_sb[:, t * SPC:(t + 1) * SPC]
                        if t <= tb:
                            nc.tensor.matmul(psA1, lhs, xt[:, c * H:c * H + 512],
                                             start=(t == 0), stop=(t == tb))
                            nc.tensor.matmul(psA2, lhs,
                                             xt[:, c * H + 512:(c + 1) * H],
                                             start=(t == 0), stop=(t == tb))
                        else:
                            nc.tensor.matmul(psB1, lhs, xt[:, c * H:c * H + 512],
                                             start=(t == tb + 1), stop=(t == NT - 1))
                            nc.tensor.matmul(psB2, lhs,
                                             xt[:, c * H + 512:(c + 1) * H],
                                             start=(t == tb + 1), stop=(t == NT - 1))
                    if g == emit_at:
                        # first-half tail, hidden under the second-half stream
                        half_ln(psA1, psA2, 0, h1cs, tph)
                if not cfg["split"]:
                    half_ln(psA1, psA2, 0, h1cs, tph)
                return psB1, psB2

            def full_pass():
                if cfg["tail"] == "none":
                    ps1 = acc.tile([SPC, 512], f32, tag="ps1")
                    ps2 = acc.tile([SPC, 256], f32, tag="ps2")
                    stream_body(ps1, ps2)
                    # minimal output so the NEFF has a data-dependent result
                    o_sb = sm.tile([SPC, 5], f32, tag="o0")
                    nc.vector.tensor_copy(o_sb, ps1[0:SPC, 0:5])
                    nc.sync.dma_start(out=out, in_=o_sb)
                    return
                # PSUM budget is 8 banks: psA1+psB1+ps2x (3) + 4 h1 chunks +
                # one shared bank ("big") for transposes, h2 and the output —
                # those three have strictly sequential lifetimes.
                h1cs = []
                for cc4 in range(4):
                    h1c = mlp.tile([128, SPC], f32, tag=f"h1c{cc4}",
                                   name=f"h1c{cc4}")
                    h1cs.append(h1c)
                big = tpp.tile([128, 64], f32, tag="big")
                tph = big[:, 0:48].rearrange("p (k j) -> p k j", k=6)
                h2v = big[:, 48:56]
                opsv = big[0:SPC, 56:61]
                psB1, psB2 = split_stream(h1cs, tph)
                half_ln(psB1, psB2, 1, h1cs, tph)
                # bias as rank-1 (b1 chunk per partition x ones over samples)
                for cc4 in range(4):
                    nc.tensor.matmul(h1cs[cc4],
                                     b1_sb[:, cc4 * 128:(cc4 + 1) * 128],
                                     ones_sb, start=False, stop=True)
                h1T = sg.tile([128, 4, SPC], xdt)
                for cc4 in range(4):
                    nc.scalar.activation(out=h1T[:, cc4, :], in_=h1cs[cc4],
                                         func=Act.Gelu)
                dbg_tiles["h1T"] = h1T

                # layer 2, transposed: h2T[128, 8] = gelu(W2^T @ h1T + b2^T)
                for k in range(4):
                    nc.tensor.matmul(h2v, w2_sb[:, k, :], h1T[:, k, :],
                                     start=(k == 0), stop=False)
                nc.tensor.matmul(h2v, b2_sb, ones_sb, start=False, stop=True)
                h2T = sg.tile([128, SPC], xdt)
                nc.scalar.activation(out=h2T, in_=h2v, func=Act.Gelu)
                dbg_tiles["h2T"] = h2T

                # layer 3: out[8, 5] = h2 @ W3 + b3
                ops = opsv
                nc.tensor.matmul(ops, h2T, w3_sb, start=True, stop=False)
                nc.tensor.matmul(ops, ones_sb, b3_sb, start=False, stop=True)
                o_sb = sm.tile([SPC, 5], f32)
                nc.vector.tensor_copy(o_sb, ops)
                nc.sync.dma_start(out=out, in_=o_sb)
                if dbg:
                    for nm, t in dbg_tiles.items():
                        if nm in dbg_out:
                            ot = sm.tile(list(dbg_out[nm].shape), f32,
                                         tag=f"dbgo_{nm}")
                            nc.vector.tensor_copy(ot, t)
                            nc.sync.dma_start(out=dbg_out[nm], in_=ot)

            if repeat == 1:
                full_pass()
            else:
                unroll = globals().get("_TIMING_UNROLL", 1)
                with tc.For_i(0, repeat, 1) as _i:
                    for _u in range(unroll):
                        full_pass()

    nc.compile()
    return nc


def _get_nc(nchunk, repeat=1, cfg=None, tb=0):
    key = (nchunk, repeat, tuple(sorted((cfg or {}).items(), key=str)), C,
           XBUFS, tb)
    if key not in _NC_CACHE:
        _NC_CACHE[key] = _build_nc(nchunk, repeat, cfg, tb)
    return _NC_CACHE[key]


def _prepare(hidden, attention_mask, gamma, beta, W1, b1, W2, b2, W3, b3):
    """Host-side sharding + packing. Returns (in_maps, core_samples, nchunk)."""
    xdt = np.dtype(XDT_NAME)
    L = attention_mask.astype(np.int64).sum(1)          # [B]
    mid = L // 2
    rows = L - 2                                        # used rows per sample

    # balance total rows across cores (greedy LPT, exactly SPC samples/core)
    order = np.argsort(-rows, kind="stable")
    core_rows = [0] * NCORES
    core_samples = [[] for _ in range(NCORES)]
    for b in order:
        cands = sorted(range(NCORES),
                       key=lambda cc: (len(core_samples[cc]) >= SPC, core_rows[cc]))
        cc = cands[0]
        core_samples[cc].append(int(b))
        core_rows[cc] += int(rows[b])

    hidden2d = np.ascontiguousarray(hidden).reshape(B * S, H)
    gamma = np.asarray(gamma, np.float64)
    beta = np.asarray(beta, np.float64)
    W1 = np.asarray(W1, np.float64)
    b1 = np.asarray(b1, np.float64)
    W1a, W1b, W1c = W1[0:H], W1[H:2 * H], W1[2 * H:3 * H]
    W1f = np.concatenate([gamma[:, None] * (W1a - W1c),
                          gamma[:, None] * (W1b + W1c)], axis=0)
    b1f = b1 + beta @ (W1a + W1b)
    shared = dict(
        idn=np.eye(16, dtype=np.float32),
        w1=W1f.astype(xdt),
        b1=b1f.astype(xdt).reshape(1, -1),
        w2=np.ascontiguousarray(W2).astype(xdt),
        b2=np.ascontiguousarray(b2).astype(xdt).reshape(1, -1),
        w3=np.ascontiguousarray(W3).astype(xdt),
        b3=np.ascontiguousarray(b3).astype(xdt).reshape(1, -1),
    )

    # A/B row layout: all first-half rows (positions [1, mid)) first, padded
    # to a common tile boundary RA_pad across cores, then second-half rows
    # ([mid, L-1)). The boundary tile index tb is compile-time-common so the
    # device can close the first-half accumulation mid-stream.
    lenA = [sum(int(mid[b]) - 1 for b in core_samples[cc]) for cc in range(NCORES)]
    lenB = [sum(int(L[b]) - 1 - int(mid[b]) for b in core_samples[cc])
            for cc in range(NCORES)]
    RA_pad = 128 * (-(-max(lenA) // 128))
    tb = RA_pad // 128 - 1
    maxrows = RA_pad + max(lenB)
    nchunk = max(1, -(-maxrows // (128 * C)))
    NT = nchunk * C
    R = NT * 128

    in_maps = []
    for cc in range(NCORES):
        samples = core_samples[cc]
        cntA = [int(mid[b]) - 1 for b in samples]
        cntB = [int(L[b]) - 1 - int(mid[b]) for b in samples]
        idxA = np.concatenate([b * S + np.arange(1, int(mid[b])) for b in samples])
        idxB = np.concatenate([b * S + np.arange(int(mid[b]), int(L[b]) - 1)
                               for b in samples])
        packed = np.zeros((R, H), xdt)
        packed[:len(idxA)] = hidden2d[idxA]
        packed[RA_pad:RA_pad + len(idxB)] = hidden2d[idxB]
        xin = np.ascontiguousarray(
            packed.reshape(nchunk, C, 128, H).transpose(0, 2, 1, 3)
            .reshape(nchunk, 128, C * H))

        m = np.zeros((R, SPC), xdt)
        colA = np.repeat(np.arange(SPC), cntA)
        colB = np.repeat(np.arange(SPC), cntB)
        m[np.arange(len(idxA)), colA] = 1.0
        m[RA_pad + np.arange(len(idxB)), colB] = 1.0
        mc = np.ascontiguousarray(
            m.reshape(NT, 128, SPC).transpose(1, 0, 2).reshape(128, NT * SPC))

        cnt1 = np.array([max(c, 1) for c in cntA], np.float64)
        cnt2 = np.array([max(c, 1) for c in cntB], np.float64)
        epsc = np.stack([1e-5 * cnt1 ** 2, 1e-5 * cnt2 ** 2], axis=1)
        epsc = epsc.astype(np.float32)  # [SPC, 2]

        in_maps.append(dict(xin=xin, mc=mc, epsc=epsc, **shared))
    return in_maps, core_samples, nchunk, tb


def kernel(**inputs):
    from concourse.bass_utils import run_bass_kernel_spmd

    args = {k: np.asarray(v) for k, v in inputs.items()}
    in_maps, core_samples, nchunk, tb = _prepare(
        args["hidden"].astype(np.float32, copy=False),
        args["attention_mask"],
        args["gamma"], args["beta"],
        args["W1"], args["b1"], args["W2"], args["b2"], args["W3"], args["b3"],
    )
    nc = _get_nc(nchunk, tb=tb)
    res = run_bass_kernel_spmd(nc, in_maps, core_ids=list(range(NCORES)))
    out = np.zeros((B, 5), np.float32)
    for cc in range(NCORES):
        o = res.results[cc]["out"]
        for j, b in enumerate(core_samples[cc]):
            out[b] = o[j]
    return out

